# revision 5
# baseline (speedup 1.0000x reference)
"""MultiHeadAttention (CLUSTERING softmax over query axis) on 8 Trainium2 cores.

Sharding: batch B=8, one batch element per NeuronCore (pure data parallel,
no collectives).

Per-core computation (L=1024, D=1024, H=16, HD=64):
  QT = (x_q @ Wq)^T            [d, l]   (bq dropped: cancels in softmax over q)
  KT = (x_k @ Wk + bk)^T       [d, l]
  V  = x_v @ Wv + bv           [l, d]
  per head h: ST_h[k, q] = QT_h . KT_h  (contraction over hd=64)
  E = exp(ST / 32)  with fused row-sums over q (free axis)
  r = 1/sums; V'_h[k, :] = V_h[k, :] * r_h[k]   (normalizer folded into V)
  OT_h[d, q] = sum_k V'_h[k, d] * E_h[k, q]
  y = OT^T @ Wo + bo           [l, d]

All matmuls in bf16 with fp32 PSUM accumulation. exp on ScalarE from PSUM.
Head pairs are packed onto the 128-wide PE array (row-tiling for scores,
col-tiling for A.V) since HD=64.
"""

import math
from contextlib import ExitStack, nullcontext

import numpy as np

import concourse.bass as bass
import concourse.tile as tile
from concourse import mybir
from concourse.bass import ts

F32 = mybir.dt.float32
BF16 = mybir.dt.bfloat16
EXP = mybir.ActivationFunctionType.Exp

L = 1024
D = 1024
P = 128
NT = 8  # 1024 / 128
N_CORES = 8
SCALE = 1.0 / math.sqrt(D)


# ---------------------------------------------------------------------------
# Workaround: this walrus build supports very few sync-wait commands per
# instruction. Tile's kernel-tail drain / barriers can carry more. Move
# excess waits onto same-engine NOPs inserted immediately before (engines
# execute their stream in order, so this preserves semantics).
def split_excess_waits(nc):
    f = nc.m.functions[0]
    ctr = 0
    for b in f.blocks:
        insts = b.instructions
        i = 0
        while i < len(insts):
            inst = insts[i]
            si = inst.sync_info
            limit = 0 if "Drain" in type(inst).__name__ else 1
            if si is not None and si.on_wait and len(si.on_wait) > limit:
                waits = list(si.on_wait)
                keep = waits[-limit:] if limit else []
                extra = waits[: len(waits) - limit]
                pos = i
                for j in range(0, len(extra), 1):
                    nop = mybir.InstNoOp(name=f"waitsplit-{ctr}", ins=[], outs=[])
                    ctr += 1
                    nop.engine = inst.engine
                    nop.bass_nofuse = True
                    nop.sync_info = mybir.SyncInfo(
                        on_wait=[extra[j]], on_update=[]
                    )
                    insts.insert(pos, nop)
                    pos += 1
                    i += 1
                inst.sync_info = mybir.SyncInfo(
                    on_wait=keep, on_update=list(si.on_update)
                )
            i += 1


# ---------------------------------------------------------------------------
def _emit_body(nc, tc, ctx, t):
    persist = ctx.enter_context(tc.tile_pool(name="persist", bufs=1))
    projp = ctx.enter_context(tc.tile_pool(name="projp", bufs=2, space="PSUM"))
    stp0 = ctx.enter_context(tc.tile_pool(name="stp0", bufs=1, space="PSUM"))
    stp1 = ctx.enter_context(tc.tile_pool(name="stp1", bufs=1, space="PSUM"))
    avp = ctx.enter_context(tc.tile_pool(name="avp", bufs=2, space="PSUM"))

    # ---- constants -------------------------------------------------------
    ones_t = persist.tile([1, P], BF16, name="ones")
    nc.vector.memset(ones_t[:], 1.0)
    bk_sb = persist.tile([P, NT], F32, name="bk")
    nc.sync.dma_start(bk_sb[:], t["bk"].rearrange("(a p) -> p a", p=P))
    bvo_f32 = persist.tile([1, 2, D], F32, name="bvo_f32")
    nc.sync.dma_start(bvo_f32[:, 0, :], t["bv"][None, :])
    nc.sync.dma_start(bvo_f32[:, 1, :], t["bo"][None, :])
    bv_bf = persist.tile([1, D], BF16, name="bv")
    bo_bf = persist.tile([1, D], BF16, name="bo")
    nc.vector.tensor_copy(bv_bf[:], bvo_f32[:, 0, :])
    nc.vector.tensor_copy(bo_bf[:], bvo_f32[:, 1, :])

    v_sb = [persist.tile([P, D], BF16, name=f"v{i}") for i in range(NT)]
    ot_sb = [persist.tile([P, D], BF16, name=f"ot{i}") for i in range(NT)]
    wo_bf = [persist.tile([P, D], BF16, name=f"wo{i}") for i in range(NT)]

    with ExitStack() as xw_ctx:
        xw = xw_ctx.enter_context(tc.tile_pool(name="xw", bufs=1))
        wq_bf = [xw.tile([P, D], BF16, name=f"wq{i}") for i in range(NT)]
        wk_bf = [xw.tile([P, D], BF16, name=f"wk{i}") for i in range(NT)]
        xqT = [xw.tile([P, D], BF16, name=f"xqT{i}") for i in range(NT)]
        xkT = [xw.tile([P, D], BF16, name=f"xkT{i}") for i in range(NT)]

        def cast_load_w(w_tiles, wdram, stage_pool):
            for i in range(NT):
                stg = stage_pool.tile([P, D], F32, name="stg")
                nc.sync.dma_start(stg[:], wdram[ts(i, P), :])
                nc.vector.tensor_copy(w_tiles[i][:], stg[:])

        def cast_transpose_x(xT_tiles, xdram, stage_pool, xbf_pool):
            for lt in range(NT):
                stg = stage_pool.tile([P, D], F32, name="stg")
                nc.sync.dma_start(stg[:], xdram[ts(lt, P), :])
                xbf = xbf_pool.tile([P, D], BF16, name="xbf")
                nc.vector.tensor_copy(xbf[:], stg[:])
                for ct in range(NT):
                    nc.sync.dma_start(
                        xT_tiles[ct][:, ts(lt, P)],
                        xbf[:, ts(ct, P)],
                        transpose=True,
                    )

        with ExitStack() as vw_ctx:
            vw = vw_ctx.enter_context(tc.tile_pool(name="vw", bufs=1))
            wv_bf = [vw.tile([P, D], BF16, name=f"wv{i}") for i in range(NT)]
            xvT = [vw.tile([P, D], BF16, name=f"xvT{i}") for i in range(NT)]
            with ExitStack() as xbf_ctx:
                stgp = xbf_ctx.enter_context(
                    tc.tile_pool(name="stgp", bufs=3)
                )
                xbfp = xbf_ctx.enter_context(
                    tc.tile_pool(name="xbfp", bufs=3)
                )
                cast_load_w(wv_bf, t["wv"], stgp)
                cast_transpose_x(xvT, t["xv"], stgp, xbfp)
                cast_load_w(wq_bf, t["wq"], stgp)
                cast_transpose_x(xqT, t["xq"], stgp, xbfp)
                cast_load_w(wk_bf, t["wk"], stgp)
                cast_transpose_x(xkT, t["xk"], stgp, xbfp)
                cast_load_w(wo_bf, t["wo"], stgp)

            # ---- V projection: V[l, d] = x_v @ Wv + bv ------------------
            for lt in range(NT):
                for dc in range(2):
                    ps = projp.tile([P, 512], F32, name="pp")
                    for ct in range(NT):
                        nc.tensor.matmul(
                            ps[:],
                            xvT[ct][:, ts(lt, P)],
                            wv_bf[ct][:, ts(dc, 512)],
                            start=(ct == 0),
                            stop=False,
                        )
                    nc.tensor.matmul(
                        ps[:],
                        ones_t[0:1, 0:P],
                        bv_bf[0:1, ts(dc, 512)],
                        start=False,
                        stop=True,
                    )
                    nc.vector.tensor_copy(v_sb[lt][:, ts(dc, 512)], ps[:])

        # ---- attention pipeline over head pairs -------------------------
        qtkt = xw_ctx.enter_context(tc.tile_pool(name="qtkt", bufs=2))
        epool = xw_ctx.enter_context(tc.tile_pool(name="epool", bufs=3))
        sums = xw_ctx.enter_context(tc.tile_pool(name="sums", bufs=4))
        vppool = xw_ctx.enter_context(tc.tile_pool(name="vppool", bufs=2))
        ypool = xw_ctx.enter_context(tc.tile_pool(name="ypool", bufs=2))

        def emit_scores(hp, qt, kt_t):
            e0 = epool.tile([P, NT, L], BF16, name="e")
            e1 = epool.tile([P, NT, L], BF16, name="e")
            s0 = sums.tile([P, NT], F32, name="esum")
            s1 = sums.tile([P, NT], F32, name="esum")
            for kt in range(NT):
                st0 = stp0.tile([P, L], F32, name="st0")
                st1 = stp1.tile([P, L], F32, name="st1")
                for qc in range(2):
                    nc.tensor.matmul(
                        st0[:, ts(qc, 512)],
                        kt_t[0:64, ts(kt, P)],
                        qt[0:64, ts(qc, 512)],
                        start=True,
                        stop=True,
                    )
                for qc in range(2):
                    nc.tensor.matmul(
                        st1[:, ts(qc, 512)],
                        kt_t[64:128, ts(kt, P)],
                        qt[64:128, ts(qc, 512)],
                        start=True,
                        stop=True,
                    )
                nc.scalar.activation(
                    e0[:, kt, :], st0[:], EXP, scale=SCALE,
                    accum_out=s0[:, kt : kt + 1],
                )
                nc.scalar.activation(
                    e1[:, kt, :], st1[:], EXP, scale=SCALE,
                    accum_out=s1[:, kt : kt + 1],
                )
            r0 = sums.tile([P, NT], F32, name="r")
            r1 = sums.tile([P, NT], F32, name="r")
            nc.vector.reciprocal(r0[:], s0[:])
            nc.vector.reciprocal(r1[:], s1[:])
            vp = vppool.tile([P, NT, P], BF16, name="vp")
            for kt in range(NT):
                nc.vector.tensor_scalar_mul(
                    vp[:, kt, 0:64],
                    v_sb[kt][:, hp * P : hp * P + 64],
                    r0[:, kt : kt + 1],
                )
                nc.vector.tensor_scalar_mul(
                    vp[:, kt, 64:128],
                    v_sb[kt][:, hp * P + 64 : hp * P + 128],
                    r1[:, kt : kt + 1],
                )
            return (hp, e0, e1, vp)

        def emit_av(prev):
            hp, e0, e1, vp = prev
            for qc in range(2):
                av = avp.tile([P, 512], F32, name="av")
                for kt in range(NT):
                    nc.tensor.matmul(
                        av[0:64, :],
                        vp[:, kt, 0:64],
                        e0[:, kt, ts(qc, 512)],
                        start=(kt == 0),
                        stop=(kt == NT - 1),
                    )
                    nc.tensor.matmul(
                        av[64:128, :],
                        vp[:, kt, 64:128],
                        e1[:, kt, ts(qc, 512)],
                        start=(kt == 0),
                        stop=(kt == NT - 1),
                    )
                nc.vector.tensor_copy(ot_sb[hp][:, ts(qc, 512)], av[:])

        prev = None
        for hp in range(NT):
            # Q projection for d-tile hp (no bias needed)
            qt = qtkt.tile([P, L], BF16, name="qt")
            for lc in range(2):
                ps = projp.tile([P, 512], F32, name="pp")
                for ct in range(NT):
                    nc.tensor.matmul(
                        ps[:],
                        wq_bf[ct][:, ts(hp, P)],
                        xqT[ct][:, ts(lc, 512)],
                        start=(ct == 0),
                        stop=(ct == NT - 1),
                    )
                nc.vector.tensor_copy(qt[:, ts(lc, 512)], ps[:])
            # K projection for d-tile hp, + bk (per-partition bias)
            kt_t = qtkt.tile([P, L], BF16, name="kt")
            for lc in range(2):
                ps = projp.tile([P, 512], F32, name="pp")
                for ct in range(NT):
                    nc.tensor.matmul(
                        ps[:],
                        wk_bf[ct][:, ts(hp, P)],
                        xkT[ct][:, ts(lc, 512)],
                        start=(ct == 0),
                        stop=(ct == NT - 1),
                    )
                nc.vector.tensor_scalar_add(
                    kt_t[:, ts(lc, 512)], ps[:], bk_sb[:, hp : hp + 1]
                )
            if prev is not None:
                emit_av(prev)
            prev = emit_scores(hp, qt, kt_t)
        emit_av(prev)

        # ---- output projection: y = OT^T @ Wo + bo ----------------------
        for lt in range(NT):
            ysb = ypool.tile([P, D], F32, name="ysb")
            for nc2 in range(2):
                ps = projp.tile([P, 512], F32, name="pp")
                for dt in range(NT):
                    nc.tensor.matmul(
                        ps[:],
                        ot_sb[dt][:, ts(lt, P)],
                        wo_bf[dt][:, ts(nc2, 512)],
                        start=(dt == 0),
                        stop=False,
                    )
                nc.tensor.matmul(
                    ps[:],
                    ones_t[0:1, 0:P],
                    bo_bf[0:1, ts(nc2, 512)],
                    start=False,
                    stop=True,
                )
                nc.vector.tensor_copy(ysb[:, ts(nc2, 512)], ps[:])
            nc.sync.dma_start(t["y"][ts(lt, P), :], ysb[:])


def build_nc(looped=False, reps=None):
    nc = bass.Bass("TRN2", debug=False, num_devices=N_CORES, num_swdge_queues=4)
    t = {}
    for name in ("xq", "xk", "xv"):
        t[name] = nc.dram_tensor(name, [L, D], F32, kind="ExternalInput")
    for name in ("wq", "wk", "wv", "wo"):
        t[name] = nc.dram_tensor(name, [D, D], F32, kind="ExternalInput")
    for name in ("bk", "bv", "bo"):
        t[name] = nc.dram_tensor(name, [D], F32, kind="ExternalInput")
    t["y"] = nc.dram_tensor("y", [L, D], F32, kind="ExternalOutput")
    with tile.TileContext(nc) as tc:
        if reps is not None:
            loop_cm = tc.For_i(0, reps, 1)
        else:
            loop_cm = nullcontext()
        with loop_cm:
            with ExitStack() as ctx:
                _emit_body(nc, tc, ctx, t)

    split_excess_waits(nc)
    return nc


# ---------------------------------------------------------------------------
# Runner: mirrors bass2jax.run_bass_via_pjrt's multi-core path, but keeps a
# reusable jitted callable (no donation) so repeated kernel() calls don't
# recompile.
def make_runner(nc, n_cores=N_CORES):
    import jax
    from jax.sharding import Mesh, NamedSharding, PartitionSpec
    from jax.experimental.shard_map import shard_map
    from concourse import bass2jax
    from concourse.bass2jax import _bass_exec_p, partition_id_tensor

    bass2jax.install_neuronx_cc_hook()

    partition_name = (
        nc.partition_id_tensor.name if nc.partition_id_tensor else None
    )
    in_names, out_names, out_avals, zero_outs = [], [], [], []
    for alloc in nc.m.functions[0].allocations:
        if not isinstance(alloc, mybir.MemoryLocationSet):
            continue
        name = alloc.memorylocations[0].name
        if alloc.kind == "ExternalInput":
            if name != partition_name:
                in_names.append(name)
        elif alloc.kind == "ExternalOutput":
            shape = tuple(alloc.tensor_shape)
            dtype = mybir.dt.np(alloc.dtype)
            out_names.append(name)
            out_avals.append(jax.core.ShapedArray(shape, dtype))
            zero_outs.append(np.zeros(shape, dtype))
    n_params = len(in_names)
    all_in_names = list(in_names) + list(out_names)
    if partition_name is not None:
        all_in_names.append(partition_name)

    def _body(*args):
        operands = list(args)
        if partition_name is not None:
            operands.append(partition_id_tensor())
        outs = _bass_exec_p.bind(
            *operands,
            out_avals=tuple(out_avals),
            in_names=tuple(all_in_names),
            out_names=tuple(out_names),
            lowering_input_output_aliases=(),
            sim_require_finite=True,
            sim_require_nnan=True,
            nc=nc,
        )
        return tuple(outs)

    devices = jax.devices()[:n_cores]
    mesh = Mesh(np.asarray(devices), ("core",))
    in_specs = (PartitionSpec("core"),) * (n_params + len(out_names))
    out_specs = (PartitionSpec("core"),) * len(out_names)
    fn = jax.jit(
        shard_map(
            _body, mesh=mesh, in_specs=in_specs, out_specs=out_specs,
            check_rep=False,
        ),
        keep_unused=True,
    )
    sharding = NamedSharding(mesh, PartitionSpec("core"))
    zeros_dev = [
        jax.device_put(
            np.zeros((n_cores * z.shape[0], *z.shape[1:]), z.dtype), sharding
        )
        for z in zero_outs
    ]

    def run(in_maps):
        per_core = [[np.asarray(m[n]) for n in in_names] for m in in_maps]
        concat_in = [
            np.concatenate([per_core[c][i] for c in range(n_cores)], axis=0)
            for i in range(n_params)
        ]
        args = [jax.device_put(a, sharding) for a in concat_in] + zeros_dev
        out = fn(*args)
        jax.block_until_ready(out)
        return [
            {
                n: np.asarray(out[i]).reshape(n_cores, *out_avals[i].shape)[c]
                for i, n in enumerate(out_names)
            }
            for c in range(n_cores)
        ]

    return run, fn, in_names, out_names, out_avals, sharding


_RUNNER = None


def _in_maps_from_inputs(inputs, looped=False, reps=1):
    maps = []
    for b in range(N_CORES):
        m = {
            "xq": np.ascontiguousarray(np.asarray(inputs["x_q"][b], np.float32)),
            "xk": np.ascontiguousarray(np.asarray(inputs["x_k"][b], np.float32)),
            "xv": np.ascontiguousarray(np.asarray(inputs["x_v"][b], np.float32)),
            "wq": np.asarray(inputs["Wq"], np.float32),
            "wk": np.asarray(inputs["Wk"], np.float32),
            "wv": np.asarray(inputs["Wv"], np.float32),
            "wo": np.asarray(inputs["Wo"], np.float32),
            "bk": np.asarray(inputs["bk"], np.float32),
            "bv": np.asarray(inputs["bv"], np.float32),
            "bo": np.asarray(inputs["bo"], np.float32),
        }
        if looped:
            m["reps"] = np.full((1, 1), reps, np.int32)
        maps.append(m)
    return maps


def kernel(**inputs) -> np.ndarray:
    global _RUNNER
    if _RUNNER is None:
        nc = build_nc(looped=False)
        _RUNNER = make_runner(nc)[0]
    in_maps = _in_maps_from_inputs(inputs)
    results = _RUNNER(in_maps)
    out = np.stack([results[b]["y"] for b in range(N_CORES)], axis=0)
    return out.astype(np.float32)


# revision 9
# speedup vs baseline: 2.0170x; 2.0170x over previous
"""MultiHeadAttention (CLUSTERING softmax over query axis) on 8 Trainium2 cores.

Sharding: batch B=8, one batch element per NeuronCore (pure data parallel,
no collectives).

Per-core computation (L=1024, D=1024, H=16, HD=64):
  QT = (x_q @ Wq)^T            [d, l]   (bq dropped: cancels in softmax over q)
  KT = (x_k @ Wk + bk)^T       [d, l]
  V  = x_v @ Wv + bv           [l, d]
  per head h: ST_h[k, q] = QT_h . KT_h  (contraction over hd=64)
  E = exp(ST / 32)  with fused row-sums over q (free axis)
  r = 1/sums; V'_h[k, :] = V_h[k, :] * r_h[k]   (normalizer folded into V)
  OT_h[d, q] = sum_k V'_h[k, d] * E_h[k, q]
  y = OT^T @ Wo + bo           [l, d]

All matmuls bf16 with fp32 PSUM accumulation. x^T via PE transpose-mode
(bf16, batched per-c-tile into one PSUM tile). exp on ScalarE from PSUM.
Head pairs are packed onto the 128-wide PE array (row-tiling for scores,
col-tiling for A.V) since HD=64.
"""

import math
from contextlib import ExitStack, nullcontext

import numpy as np

import concourse.bass as bass
import concourse.tile as tile
from concourse import mybir
from concourse.bass import ts

F32 = mybir.dt.float32
BF16 = mybir.dt.bfloat16
EXP = mybir.ActivationFunctionType.Exp
COPY = mybir.ActivationFunctionType.Copy

L = 1024
D = 1024
P = 128
NT = 8  # 1024 / 128
N_CORES = 8
SCALE = 1.0 / math.sqrt(D)


# ---------------------------------------------------------------------------
# Workaround: this walrus build supports very few sync-wait commands per
# instruction. Tile's kernel-tail drain / barriers can carry more. Move
# excess waits onto same-engine NOPs inserted immediately before (engines
# execute their stream in order, so this preserves semantics).
def split_excess_waits(nc):
    f = nc.m.functions[0]
    ctr = 0
    for b in f.blocks:
        insts = b.instructions
        i = 0
        while i < len(insts):
            inst = insts[i]
            si = inst.sync_info
            limit = 0 if "Drain" in type(inst).__name__ else 1
            if si is not None and si.on_wait and len(si.on_wait) > limit:
                waits = list(si.on_wait)
                keep = waits[-limit:] if limit else []
                extra = waits[: len(waits) - limit]
                pos = i
                for j in range(0, len(extra), 1):
                    nop = mybir.InstNoOp(name=f"waitsplit-{ctr}", ins=[], outs=[])
                    ctr += 1
                    nop.engine = inst.engine
                    nop.bass_nofuse = True
                    nop.sync_info = mybir.SyncInfo(
                        on_wait=[extra[j]], on_update=[]
                    )
                    insts.insert(pos, nop)
                    pos += 1
                    i += 1
                inst.sync_info = mybir.SyncInfo(
                    on_wait=keep, on_update=list(si.on_update)
                )
            i += 1


# ---------------------------------------------------------------------------
def _emit_body(nc, tc, ctx, t):
    persist = ctx.enter_context(tc.tile_pool(name="persist", bufs=1))
    projp = ctx.enter_context(tc.tile_pool(name="projp", bufs=2, space="PSUM"))

    # ---- constants -------------------------------------------------------
    ones_t = persist.tile([1, P], BF16, name="ones")
    nc.vector.memset(ones_t[:], 1.0)
    ident = persist.tile([P, P], BF16, name="ident")
    nc.sync.dma_start(ident[:], t["ident"][:, :])
    bk_sb = persist.tile([P, NT], F32, name="bk")
    nc.sync.dma_start(bk_sb[:], t["bk"].rearrange("(a p) -> p a", p=P))
    bv_bf = persist.tile([1, D], BF16, name="bv")
    bo_bf = persist.tile([1, D], BF16, name="bo")

    v_sb = [persist.tile([P, D], BF16, name=f"v{i}") for i in range(NT)]
    ot_sb = [persist.tile([P, D], BF16, name=f"ot{i}") for i in range(NT)]
    wo_bf = [persist.tile([P, D], BF16, name=f"wo{i}") for i in range(NT)]

    with ExitStack() as xw_ctx:
        xw = xw_ctx.enter_context(tc.tile_pool(name="xw", bufs=1))
        wq_bf = [xw.tile([P, D], BF16, name=f"wq{i}") for i in range(NT)]
        wk_bf = [xw.tile([P, D], BF16, name=f"wk{i}") for i in range(NT)]
        xqT = [xw.tile([P, D], BF16, name=f"xqT{i}") for i in range(NT)]
        xkT = [xw.tile([P, D], BF16, name=f"xkT{i}") for i in range(NT)]

        with ExitStack() as vw_ctx:
            vw = vw_ctx.enter_context(tc.tile_pool(name="vw", bufs=1))
            wv_bf = [vw.tile([P, D], BF16, name=f"wv{i}") for i in range(NT)]
            xvT = [vw.tile([P, D], BF16, name=f"xvT{i}") for i in range(NT)]

            with ExitStack() as ph_a:
                stgp = ph_a.enter_context(tc.tile_pool(name="stgp", bufs=3))
                xbfp = ph_a.enter_context(tc.tile_pool(name="xbfp", bufs=1))
                tpp = ph_a.enter_context(
                    tc.tile_pool(name="tpp", bufs=2, space="PSUM")
                )

                bstg = stgp.tile([P, D], F32, name="stg")
                nc.sync.dma_start(bstg[0:1, :], t["bv"][None, :])
                nc.vector.tensor_copy(bv_bf[:], bstg[0:1, :])
                bstg2 = stgp.tile([P, D], F32, name="stg")
                nc.sync.dma_start(bstg2[0:1, :], t["bo"][None, :])
                nc.vector.tensor_copy(bo_bf[:], bstg2[0:1, :])

                def wpath(w_tiles, wdram):
                    for i in range(NT):
                        stg = stgp.tile([P, D], F32, name="stg")
                        nc.sync.dma_start(stg[:], wdram[ts(i, P), :])
                        nc.vector.tensor_copy(w_tiles[i][:], stg[:])

                def xpath(xT_tiles, xdram, act_copies):
                    xbfs = []
                    for lt in range(NT):
                        stg = stgp.tile([P, D], F32, name="stg")
                        nc.sync.dma_start(stg[:], xdram[ts(lt, P), :])
                        xbf = xbfp.tile([P, D], BF16, name=f"xbf{lt}")
                        nc.vector.tensor_copy(xbf[:], stg[:])
                        xbfs.append(xbf)
                    for ct in range(NT):
                        tp = tpp.tile([P, D], BF16, name="tp")
                        for lt in range(NT):
                            nc.tensor.transpose(
                                tp[:, ts(lt, P)],
                                xbfs[lt][:, ts(ct, P)],
                                ident[:],
                            )
                        if act_copies:
                            nc.scalar.activation(xT_tiles[ct][:], tp[:], COPY)
                        else:
                            nc.vector.tensor_copy(xT_tiles[ct][:], tp[:])

                # V path first: its projection fills the PE while q/k load.
                xpath(xvT, t["xv"], act_copies=True)
                wpath(wv_bf, t["wv"])

                # ---- V projection: V[l, d] = x_v @ Wv + bv --------------
                for lt in range(NT):
                    ps = [projp.tile([P, 512], F32, name="pp") for _ in range(2)]
                    for ct in range(NT):
                        for dc in range(2):
                            nc.tensor.matmul(
                                ps[dc][:],
                                xvT[ct][:, ts(lt, P)],
                                wv_bf[ct][:, ts(dc, 512)],
                                start=(ct == 0),
                                stop=False,
                            )
                    for dc in range(2):
                        nc.tensor.matmul(
                            ps[dc][:],
                            ones_t[0:1, 0:P],
                            bv_bf[0:1, ts(dc, 512)],
                            start=False,
                            stop=True,
                        )
                        nc.vector.tensor_copy(v_sb[lt][:, ts(dc, 512)], ps[dc][:])

                xpath(xqT, t["xq"], act_copies=True)
                wpath(wq_bf, t["wq"])
                xpath(xkT, t["xk"], act_copies=True)
                wpath(wk_bf, t["wk"])
                wpath(wo_bf, t["wo"])

        # ---- attention pipeline over head pairs -------------------------
        qtkt = xw_ctx.enter_context(tc.tile_pool(name="qtkt", bufs=2))
        epool = xw_ctx.enter_context(tc.tile_pool(name="epool", bufs=4))
        sums = xw_ctx.enter_context(tc.tile_pool(name="sums", bufs=4))
        vppool = xw_ctx.enter_context(tc.tile_pool(name="vppool", bufs=2))
        ypool = xw_ctx.enter_context(tc.tile_pool(name="ypool", bufs=2))
        stp0 = xw_ctx.enter_context(tc.tile_pool(name="stp0", bufs=1, space="PSUM"))
        stp1 = xw_ctx.enter_context(tc.tile_pool(name="stp1", bufs=1, space="PSUM"))
        avp = xw_ctx.enter_context(tc.tile_pool(name="avp", bufs=1, space="PSUM"))

        def emit_scores(hp, qt, kt_t):
            e0 = epool.tile([P, NT, L], BF16, name="e")
            e1 = epool.tile([P, NT, L], BF16, name="e")
            s0 = sums.tile([P, NT], F32, name="esum")
            s1 = sums.tile([P, NT], F32, name="esum")
            for kt in range(NT):
                st0 = stp0.tile([P, L], F32, name="st0")
                st1 = stp1.tile([P, L], F32, name="st1")
                for qc in range(2):
                    nc.tensor.matmul(
                        st0[:, ts(qc, 512)],
                        kt_t[0:64, ts(kt, P)],
                        qt[0:64, ts(qc, 512)],
                        start=True,
                        stop=True,
                    )
                for qc in range(2):
                    nc.tensor.matmul(
                        st1[:, ts(qc, 512)],
                        kt_t[64:128, ts(kt, P)],
                        qt[64:128, ts(qc, 512)],
                        start=True,
                        stop=True,
                    )
                nc.scalar.activation(
                    e0[:, kt, :], st0[:], EXP, scale=SCALE,
                    accum_out=s0[:, kt : kt + 1],
                )
                nc.scalar.activation(
                    e1[:, kt, :], st1[:], EXP, scale=SCALE,
                    accum_out=s1[:, kt : kt + 1],
                )
            r0 = sums.tile([P, NT], F32, name="r")
            r1 = sums.tile([P, NT], F32, name="r")
            nc.vector.reciprocal(r0[:], s0[:])
            nc.vector.reciprocal(r1[:], s1[:])
            vp = vppool.tile([P, NT, P], BF16, name="vp")
            for kt in range(NT):
                nc.vector.tensor_scalar_mul(
                    vp[:, kt, 0:64],
                    v_sb[kt][:, hp * P : hp * P + 64],
                    r0[:, kt : kt + 1],
                )
                nc.vector.tensor_scalar_mul(
                    vp[:, kt, 64:128],
                    v_sb[kt][:, hp * P + 64 : hp * P + 128],
                    r1[:, kt : kt + 1],
                )
            return (hp, e0, e1, vp)

        def emit_av(prev):
            # One PSUM bank per accumulation group: pending groups must not
            # share a bank (start_tensor_calc zeroes the whole zero-region).
            hp, e0, e1, vp = prev
            for qc in range(2):
                avA = avp.tile([P, 512], F32, name="avA")
                avB = avp.tile([P, 512], F32, name="avB")
                for kt in range(NT):
                    nc.tensor.matmul(
                        avA[0:64, :],
                        vp[:, kt, 0:64],
                        e0[:, kt, ts(qc, 512)],
                        start=(kt == 0),
                        stop=(kt == NT - 1),
                    )
                    nc.tensor.matmul(
                        avB[64:128, :],
                        vp[:, kt, 64:128],
                        e1[:, kt, ts(qc, 512)],
                        start=(kt == 0),
                        stop=(kt == NT - 1),
                    )
                nc.vector.tensor_copy(
                    ot_sb[hp][0:64, ts(qc, 512)], avA[0:64, :]
                )
                nc.vector.tensor_copy(
                    ot_sb[hp][64:128, ts(qc, 512)], avB[64:128, :]
                )

        def emit_proj(hp, w_bf, xT, out_tag):
            out_t = qtkt.tile([P, L], BF16, name=out_tag)
            ps = [projp.tile([P, 512], F32, name="pp") for _ in range(2)]
            for ct in range(NT):
                for lc in range(2):
                    nc.tensor.matmul(
                        ps[lc][:],
                        w_bf[ct][:, ts(hp, P)],
                        xT[ct][:, ts(lc, 512)],
                        start=(ct == 0),
                        stop=(ct == NT - 1),
                    )
            for lc in range(2):
                if out_tag == "kt":
                    nc.vector.tensor_scalar_add(
                        out_t[:, ts(lc, 512)], ps[lc][:], bk_sb[:, hp : hp + 1]
                    )
                else:
                    nc.vector.tensor_copy(out_t[:, ts(lc, 512)], ps[lc][:])
            return out_t

        prev = None
        for hp in range(NT):
            qt = emit_proj(hp, wq_bf, xqT, "qt")
            kt_t = emit_proj(hp, wk_bf, xkT, "kt")
            if prev is not None:
                emit_av(prev)
            prev = emit_scores(hp, qt, kt_t)
        emit_av(prev)

        # ---- output projection: y = OT^T @ Wo + bo ----------------------
        for lt in range(NT):
            ysb = ypool.tile([P, D], F32, name="ysb")
            ps = [projp.tile([P, 512], F32, name="pp") for _ in range(2)]
            for dt in range(NT):
                for nc2 in range(2):
                    nc.tensor.matmul(
                        ps[nc2][:],
                        ot_sb[dt][:, ts(lt, P)],
                        wo_bf[dt][:, ts(nc2, 512)],
                        start=(dt == 0),
                        stop=False,
                    )
            for nc2 in range(2):
                nc.tensor.matmul(
                    ps[nc2][:],
                    ones_t[0:1, 0:P],
                    bo_bf[0:1, ts(nc2, 512)],
                    start=False,
                    stop=True,
                )
                nc.vector.tensor_copy(ysb[:, ts(nc2, 512)], ps[nc2][:])
            nc.sync.dma_start(t["y"][ts(lt, P), :], ysb[:])


def build_nc(looped=False, reps=None, do_split=True):
    nc = bass.Bass("TRN2", debug=False, num_devices=N_CORES, num_swdge_queues=4)
    t = {}
    for name in ("xq", "xk", "xv"):
        t[name] = nc.dram_tensor(name, [L, D], F32, kind="ExternalInput")
    for name in ("wq", "wk", "wv", "wo"):
        t[name] = nc.dram_tensor(name, [D, D], F32, kind="ExternalInput")
    for name in ("bk", "bv", "bo"):
        t[name] = nc.dram_tensor(name, [D], F32, kind="ExternalInput")
    t["ident"] = nc.dram_tensor("ident", [P, P], BF16, kind="ExternalInput")
    t["y"] = nc.dram_tensor("y", [L, D], F32, kind="ExternalOutput")

    with tile.TileContext(nc) as tc:
        if reps is not None:
            loop_cm = tc.For_i(0, reps, 1)
        else:
            loop_cm = nullcontext()
        with loop_cm:
            with ExitStack() as ctx:
                _emit_body(nc, tc, ctx, t)

    if do_split:
        split_excess_waits(nc)
    return nc


# ---------------------------------------------------------------------------
# Runner: mirrors bass2jax.run_bass_via_pjrt's multi-core path, but keeps a
# reusable jitted callable (no donation) so repeated kernel() calls don't
# recompile.
def make_runner(nc, n_cores=N_CORES):
    import jax
    from jax.sharding import Mesh, NamedSharding, PartitionSpec
    from jax.experimental.shard_map import shard_map
    from concourse import bass2jax
    from concourse.bass2jax import _bass_exec_p, partition_id_tensor

    bass2jax.install_neuronx_cc_hook()

    partition_name = (
        nc.partition_id_tensor.name if nc.partition_id_tensor else None
    )
    in_names, out_names, out_avals, zero_outs = [], [], [], []
    for alloc in nc.m.functions[0].allocations:
        if not isinstance(alloc, mybir.MemoryLocationSet):
            continue
        name = alloc.memorylocations[0].name
        if alloc.kind == "ExternalInput":
            if name != partition_name:
                in_names.append(name)
        elif alloc.kind == "ExternalOutput":
            shape = tuple(alloc.tensor_shape)
            dtype = mybir.dt.np(alloc.dtype)
            out_names.append(name)
            out_avals.append(jax.core.ShapedArray(shape, dtype))
            zero_outs.append(np.zeros(shape, dtype))
    n_params = len(in_names)
    all_in_names = list(in_names) + list(out_names)
    if partition_name is not None:
        all_in_names.append(partition_name)

    def _body(*args):
        operands = list(args)
        if partition_name is not None:
            operands.append(partition_id_tensor())
        outs = _bass_exec_p.bind(
            *operands,
            out_avals=tuple(out_avals),
            in_names=tuple(all_in_names),
            out_names=tuple(out_names),
            lowering_input_output_aliases=(),
            sim_require_finite=True,
            sim_require_nnan=True,
            nc=nc,
        )
        return tuple(outs)

    devices = jax.devices()[:n_cores]
    mesh = Mesh(np.asarray(devices), ("core",))
    in_specs = (PartitionSpec("core"),) * (n_params + len(out_names))
    out_specs = (PartitionSpec("core"),) * len(out_names)
    fn = jax.jit(
        shard_map(
            _body, mesh=mesh, in_specs=in_specs, out_specs=out_specs,
            check_rep=False,
        ),
        keep_unused=True,
    )
    sharding = NamedSharding(mesh, PartitionSpec("core"))
    zeros_dev = [
        jax.device_put(
            np.zeros((n_cores * z.shape[0], *z.shape[1:]), z.dtype), sharding
        )
        for z in zero_outs
    ]

    def run(in_maps):
        per_core = [[np.asarray(m[n]) for n in in_names] for m in in_maps]
        concat_in = [
            np.concatenate([per_core[c][i] for c in range(n_cores)], axis=0)
            for i in range(n_params)
        ]
        args = [jax.device_put(a, sharding) for a in concat_in] + zeros_dev
        out = fn(*args)
        jax.block_until_ready(out)
        return [
            {
                n: np.asarray(out[i]).reshape(n_cores, *out_avals[i].shape)[c]
                for i, n in enumerate(out_names)
            }
            for c in range(n_cores)
        ]

    return run, fn, in_names, out_names, out_avals, sharding


_RUNNER = None


def _in_maps_from_inputs(inputs):
    import ml_dtypes

    ident = np.eye(P, dtype=ml_dtypes.bfloat16)
    maps = []
    for b in range(N_CORES):
        m = {
            "xq": np.ascontiguousarray(np.asarray(inputs["x_q"][b], np.float32)),
            "xk": np.ascontiguousarray(np.asarray(inputs["x_k"][b], np.float32)),
            "xv": np.ascontiguousarray(np.asarray(inputs["x_v"][b], np.float32)),
            "wq": np.asarray(inputs["Wq"], np.float32),
            "wk": np.asarray(inputs["Wk"], np.float32),
            "wv": np.asarray(inputs["Wv"], np.float32),
            "wo": np.asarray(inputs["Wo"], np.float32),
            "bk": np.asarray(inputs["bk"], np.float32),
            "bv": np.asarray(inputs["bv"], np.float32),
            "bo": np.asarray(inputs["bo"], np.float32),
            "ident": ident,
        }
        maps.append(m)
    return maps


def kernel(**inputs) -> np.ndarray:
    global _RUNNER
    if _RUNNER is None:
        nc = build_nc()
        _RUNNER = make_runner(nc)[0]
    in_maps = _in_maps_from_inputs(inputs)
    results = _RUNNER(in_maps)
    out = np.stack([results[b]["y"] for b in range(N_CORES)], axis=0)
    return out.astype(np.float32)


# revision 10
# speedup vs baseline: 2.0476x; 1.0152x over previous
"""MultiHeadAttention (CLUSTERING softmax over query axis) on 8 Trainium2 cores.

Sharding: batch B=8, one batch element per NeuronCore (pure data parallel,
no collectives).

Per-core computation (L=1024, D=1024, H=16, HD=64):
  QT = (x_q @ Wq)^T            [d, l]   (bq dropped: cancels in softmax over q)
  KT = (x_k @ Wk + bk)^T       [d, l]
  V  = x_v @ Wv + bv           [l, d]
  per head h: ST_h[k, q] = QT_h . KT_h  (contraction over hd=64)
  E = exp(ST / 32)  with fused row-sums over q (free axis)
  r = 1/sums; V'_h[k, :] = V_h[k, :] * r_h[k]   (normalizer folded into V)
  OT_h[d, q] = sum_k V'_h[k, d] * E_h[k, q]
  y = OT^T @ Wo + bo           [l, d]

All matmuls bf16 with fp32 PSUM accumulation. x^T via PE transpose-mode
(bf16, batched per-c-tile into one PSUM tile). exp on ScalarE from PSUM.
Head pairs are packed onto the 128-wide PE array (row-tiling for scores,
col-tiling for A.V) since HD=64.
"""

import math
from contextlib import ExitStack, nullcontext

import numpy as np

import concourse.bass as bass
import concourse.tile as tile
from concourse import mybir
from concourse.bass import ts

F32 = mybir.dt.float32
BF16 = mybir.dt.bfloat16
EXP = mybir.ActivationFunctionType.Exp
COPY = mybir.ActivationFunctionType.Copy

L = 1024
D = 1024
P = 128
NT = 8  # 1024 / 128
N_CORES = 8
SCALE = 1.0 / math.sqrt(D)


# ---------------------------------------------------------------------------
# Workaround: this walrus build supports very few sync-wait commands per
# instruction. Tile's kernel-tail drain / barriers can carry more. Move
# excess waits onto same-engine NOPs inserted immediately before (engines
# execute their stream in order, so this preserves semantics).
def split_excess_waits(nc):
    f = nc.m.functions[0]
    ctr = 0
    for b in f.blocks:
        insts = b.instructions
        i = 0
        while i < len(insts):
            inst = insts[i]
            si = inst.sync_info
            limit = 0 if "Drain" in type(inst).__name__ else 1
            if si is not None and si.on_wait and len(si.on_wait) > limit:
                waits = list(si.on_wait)
                keep = waits[-limit:] if limit else []
                extra = waits[: len(waits) - limit]
                pos = i
                for j in range(0, len(extra), 1):
                    nop = mybir.InstNoOp(name=f"waitsplit-{ctr}", ins=[], outs=[])
                    ctr += 1
                    nop.engine = inst.engine
                    nop.bass_nofuse = True
                    nop.sync_info = mybir.SyncInfo(
                        on_wait=[extra[j]], on_update=[]
                    )
                    insts.insert(pos, nop)
                    pos += 1
                    i += 1
                inst.sync_info = mybir.SyncInfo(
                    on_wait=keep, on_update=list(si.on_update)
                )
            i += 1


# ---------------------------------------------------------------------------
def _emit_body(nc, tc, ctx, t):
    persist = ctx.enter_context(tc.tile_pool(name="persist", bufs=1))
    projp = ctx.enter_context(tc.tile_pool(name="projp", bufs=2, space="PSUM"))

    # ---- constants -------------------------------------------------------
    ones_t = persist.tile([1, P], BF16, name="ones")
    nc.vector.memset(ones_t[:], 1.0)
    bk_sb = persist.tile([P, NT], F32, name="bk")
    nc.sync.dma_start(bk_sb[:], t["bk"].rearrange("(a p) -> p a", p=P))
    bo_bf = persist.tile([1, D], BF16, name="bo")

    v_sb = [persist.tile([P, D], BF16, name=f"v{i}") for i in range(NT)]
    ot_sb = [persist.tile([P, D], BF16, name=f"ot{i}") for i in range(NT)]
    wo_bf = [persist.tile([P, D], BF16, name=f"wo{i}") for i in range(NT)]

    with ExitStack() as xw_ctx:
        xw = xw_ctx.enter_context(tc.tile_pool(name="xw", bufs=1))
        wq_bf = [xw.tile([P, D], BF16, name=f"wq{i}") for i in range(NT)]
        wk_bf = [xw.tile([P, D], BF16, name=f"wk{i}") for i in range(NT)]
        xqT = [xw.tile([P, D], BF16, name=f"xqT{i}") for i in range(NT)]
        xkT = [xw.tile([P, D], BF16, name=f"xkT{i}") for i in range(NT)]

        with ExitStack() as vw_ctx:
            vw = vw_ctx.enter_context(tc.tile_pool(name="vw", bufs=1))
            wv_bf = [vw.tile([P, D], BF16, name=f"wv{i}") for i in range(NT)]
            xvT = [vw.tile([P, D], BF16, name=f"xvT{i}") for i in range(NT)]
            ident = vw.tile([P, P], BF16, name="ident")
            nc.sync.dma_start(ident[:], t["ident"][:, :])
            bv_bf = vw.tile([1, D], BF16, name="bv")

            with ExitStack() as ph_a:
                stgp = ph_a.enter_context(tc.tile_pool(name="stgp", bufs=5))
                xbfp = ph_a.enter_context(tc.tile_pool(name="xbfp", bufs=1))
                tpp = ph_a.enter_context(
                    tc.tile_pool(name="tpp", bufs=2, space="PSUM")
                )

                bstg = stgp.tile([P, D], F32, name="stg")
                nc.sync.dma_start(bstg[0:1, :], t["bv"][None, :])
                nc.vector.tensor_copy(bv_bf[:], bstg[0:1, :])
                bstg2 = stgp.tile([P, D], F32, name="stg")
                nc.sync.dma_start(bstg2[0:1, :], t["bo"][None, :])
                nc.vector.tensor_copy(bo_bf[:], bstg2[0:1, :])

                def wpath(w_tiles, wdram):
                    for i in range(NT):
                        stg = stgp.tile([P, D], F32, name="stg")
                        nc.sync.dma_start(stg[:], wdram[ts(i, P), :])
                        nc.vector.tensor_copy(w_tiles[i][:], stg[:])

                def xpath(xT_tiles, xdram, act_copies):
                    xbfs = []
                    for lt in range(NT):
                        stg = stgp.tile([P, D], F32, name="stg")
                        nc.sync.dma_start(stg[:], xdram[ts(lt, P), :])
                        xbf = xbfp.tile([P, D], BF16, name=f"xbf{lt}")
                        nc.vector.tensor_copy(xbf[:], stg[:])
                        xbfs.append(xbf)
                    for ct in range(NT):
                        tp = tpp.tile([P, D], BF16, name="tp")
                        for lt in range(NT):
                            nc.tensor.transpose(
                                tp[:, ts(lt, P)],
                                xbfs[lt][:, ts(ct, P)],
                                ident[:],
                            )
                        if act_copies:
                            nc.scalar.activation(xT_tiles[ct][:], tp[:], COPY)
                        else:
                            nc.vector.tensor_copy(xT_tiles[ct][:], tp[:])

                # V path first: its projection fills the PE while q/k load.
                xpath(xvT, t["xv"], act_copies=True)
                wpath(wv_bf, t["wv"])

                # ---- V projection: V[l, d] = x_v @ Wv + bv --------------
                for lt in range(NT):
                    ps = [projp.tile([P, 512], F32, name="pp") for _ in range(2)]
                    for ct in range(NT):
                        for dc in range(2):
                            nc.tensor.matmul(
                                ps[dc][:],
                                xvT[ct][:, ts(lt, P)],
                                wv_bf[ct][:, ts(dc, 512)],
                                start=(ct == 0),
                                stop=False,
                            )
                    for dc in range(2):
                        nc.tensor.matmul(
                            ps[dc][:],
                            ones_t[0:1, 0:P],
                            bv_bf[0:1, ts(dc, 512)],
                            start=False,
                            stop=True,
                        )
                        nc.vector.tensor_copy(v_sb[lt][:, ts(dc, 512)], ps[dc][:])

                xpath(xqT, t["xq"], act_copies=True)
                wpath(wq_bf, t["wq"])
                xpath(xkT, t["xk"], act_copies=True)
                wpath(wk_bf, t["wk"])
                wpath(wo_bf, t["wo"])

        # ---- attention pipeline over head pairs -------------------------
        qtkt = xw_ctx.enter_context(tc.tile_pool(name="qtkt", bufs=2))
        epool = xw_ctx.enter_context(tc.tile_pool(name="epool", bufs=3))
        sums = xw_ctx.enter_context(tc.tile_pool(name="sums", bufs=4))
        vppool = xw_ctx.enter_context(tc.tile_pool(name="vppool", bufs=2))
        ypool = xw_ctx.enter_context(tc.tile_pool(name="ypool", bufs=1))
        stp0 = xw_ctx.enter_context(tc.tile_pool(name="stp0", bufs=1, space="PSUM"))
        stp1 = xw_ctx.enter_context(tc.tile_pool(name="stp1", bufs=1, space="PSUM"))
        avp = xw_ctx.enter_context(tc.tile_pool(name="avp", bufs=1, space="PSUM"))

        def emit_scores(hp, qt, kt_t):
            e0 = epool.tile([P, NT, L], BF16, name="e")
            e1 = epool.tile([P, NT, L], BF16, name="e")
            s0 = sums.tile([P, NT], F32, name="esum")
            s1 = sums.tile([P, NT], F32, name="esum")
            for kt in range(NT):
                st0 = stp0.tile([P, L], F32, name="st0")
                st1 = stp1.tile([P, L], F32, name="st1")
                for qc in range(2):
                    nc.tensor.matmul(
                        st0[:, ts(qc, 512)],
                        kt_t[0:64, ts(kt, P)],
                        qt[0:64, ts(qc, 512)],
                        start=True,
                        stop=True,
                    )
                for qc in range(2):
                    nc.tensor.matmul(
                        st1[:, ts(qc, 512)],
                        kt_t[64:128, ts(kt, P)],
                        qt[64:128, ts(qc, 512)],
                        start=True,
                        stop=True,
                    )
                nc.scalar.activation(
                    e0[:, kt, :], st0[:], EXP, scale=SCALE,
                    accum_out=s0[:, kt : kt + 1],
                )
                nc.scalar.activation(
                    e1[:, kt, :], st1[:], EXP, scale=SCALE,
                    accum_out=s1[:, kt : kt + 1],
                )
            r0 = sums.tile([P, NT], F32, name="r")
            r1 = sums.tile([P, NT], F32, name="r")
            nc.vector.reciprocal(r0[:], s0[:])
            nc.vector.reciprocal(r1[:], s1[:])
            vp = vppool.tile([P, NT, P], BF16, name="vp")
            for kt in range(NT):
                nc.vector.tensor_scalar_mul(
                    vp[:, kt, 0:64],
                    v_sb[kt][:, hp * P : hp * P + 64],
                    r0[:, kt : kt + 1],
                )
                nc.vector.tensor_scalar_mul(
                    vp[:, kt, 64:128],
                    v_sb[kt][:, hp * P + 64 : hp * P + 128],
                    r1[:, kt : kt + 1],
                )
            return (hp, e0, e1, vp)

        def emit_av(prev):
            # One PSUM bank per accumulation group: pending groups must not
            # share a bank (start_tensor_calc zeroes the whole zero-region).
            hp, e0, e1, vp = prev
            for qc in range(2):
                avA = avp.tile([P, 512], F32, name="avA")
                avB = avp.tile([P, 512], F32, name="avB")
                for kt in range(NT):
                    nc.tensor.matmul(
                        avA[0:64, :],
                        vp[:, kt, 0:64],
                        e0[:, kt, ts(qc, 512)],
                        start=(kt == 0),
                        stop=(kt == NT - 1),
                    )
                    nc.tensor.matmul(
                        avB[64:128, :],
                        vp[:, kt, 64:128],
                        e1[:, kt, ts(qc, 512)],
                        start=(kt == 0),
                        stop=(kt == NT - 1),
                    )
                nc.vector.tensor_copy(
                    ot_sb[hp][0:64, ts(qc, 512)], avA[0:64, :]
                )
                nc.vector.tensor_copy(
                    ot_sb[hp][64:128, ts(qc, 512)], avB[64:128, :]
                )

        def emit_proj(hp, w_bf, xT, out_tag):
            out_t = qtkt.tile([P, L], BF16, name=out_tag)
            ps = [projp.tile([P, 512], F32, name="pp") for _ in range(2)]
            for ct in range(NT):
                for lc in range(2):
                    nc.tensor.matmul(
                        ps[lc][:],
                        w_bf[ct][:, ts(hp, P)],
                        xT[ct][:, ts(lc, 512)],
                        start=(ct == 0),
                        stop=(ct == NT - 1),
                    )
            for lc in range(2):
                if out_tag == "kt":
                    nc.vector.tensor_scalar_add(
                        out_t[:, ts(lc, 512)], ps[lc][:], bk_sb[:, hp : hp + 1]
                    )
                else:
                    nc.vector.tensor_copy(out_t[:, ts(lc, 512)], ps[lc][:])
            return out_t

        ypart = [ypool.tile([P, D], F32, name=f"yp{i}") for i in range(NT)]

        def outproj_batch1():
            # contract pairs 0..6 into y partials while pair 7 is in flight
            for lt in range(NT):
                ps = [projp.tile([P, 512], F32, name="pp") for _ in range(2)]
                for dt in range(NT - 1):
                    for nc2 in range(2):
                        nc.tensor.matmul(
                            ps[nc2][:],
                            ot_sb[dt][:, ts(lt, P)],
                            wo_bf[dt][:, ts(nc2, 512)],
                            start=(dt == 0),
                            stop=(dt == NT - 2),
                        )
                for nc2 in range(2):
                    nc.vector.tensor_copy(
                        ypart[lt][:, ts(nc2, 512)], ps[nc2][:]
                    )

        def outproj_batch2():
            for lt in range(NT):
                ps = [projp.tile([P, 512], F32, name="pp") for _ in range(2)]
                for nc2 in range(2):
                    nc.tensor.matmul(
                        ps[nc2][:],
                        ot_sb[NT - 1][:, ts(lt, P)],
                        wo_bf[NT - 1][:, ts(nc2, 512)],
                        start=True,
                        stop=False,
                    )
                    nc.tensor.matmul(
                        ps[nc2][:],
                        ones_t[0:1, 0:P],
                        bo_bf[0:1, ts(nc2, 512)],
                        start=False,
                        stop=True,
                    )
                for nc2 in range(2):
                    nc.vector.tensor_tensor(
                        ypart[lt][:, ts(nc2, 512)],
                        ps[nc2][:],
                        ypart[lt][:, ts(nc2, 512)],
                        mybir.AluOpType.add,
                    )
                nc.sync.dma_start(t["y"][ts(lt, P), :], ypart[lt][:])

        prev = None
        for hp in range(NT):
            qt = emit_proj(hp, wq_bf, xqT, "qt")
            kt_t = emit_proj(hp, wk_bf, xkT, "kt")
            if prev is not None:
                emit_av(prev)
            if hp == NT - 1:
                outproj_batch1()
            prev = emit_scores(hp, qt, kt_t)
        emit_av(prev)
        outproj_batch2()


def build_nc(looped=False, reps=None, do_split=True):
    nc = bass.Bass("TRN2", debug=False, num_devices=N_CORES, num_swdge_queues=4)
    t = {}
    for name in ("xq", "xk", "xv"):
        t[name] = nc.dram_tensor(name, [L, D], F32, kind="ExternalInput")
    for name in ("wq", "wk", "wv", "wo"):
        t[name] = nc.dram_tensor(name, [D, D], F32, kind="ExternalInput")
    for name in ("bk", "bv", "bo"):
        t[name] = nc.dram_tensor(name, [D], F32, kind="ExternalInput")
    t["ident"] = nc.dram_tensor("ident", [P, P], BF16, kind="ExternalInput")
    t["y"] = nc.dram_tensor("y", [L, D], F32, kind="ExternalOutput")

    with tile.TileContext(nc) as tc:
        if reps is not None:
            loop_cm = tc.For_i(0, reps, 1)
        else:
            loop_cm = nullcontext()
        with loop_cm:
            with ExitStack() as ctx:
                _emit_body(nc, tc, ctx, t)

    if do_split:
        split_excess_waits(nc)
    return nc


# ---------------------------------------------------------------------------
# Runner: mirrors bass2jax.run_bass_via_pjrt's multi-core path, but keeps a
# reusable jitted callable (no donation) so repeated kernel() calls don't
# recompile.
def make_runner(nc, n_cores=N_CORES):
    import jax
    from jax.sharding import Mesh, NamedSharding, PartitionSpec
    from jax.experimental.shard_map import shard_map
    from concourse import bass2jax
    from concourse.bass2jax import _bass_exec_p, partition_id_tensor

    bass2jax.install_neuronx_cc_hook()

    partition_name = (
        nc.partition_id_tensor.name if nc.partition_id_tensor else None
    )
    in_names, out_names, out_avals, zero_outs = [], [], [], []
    for alloc in nc.m.functions[0].allocations:
        if not isinstance(alloc, mybir.MemoryLocationSet):
            continue
        name = alloc.memorylocations[0].name
        if alloc.kind == "ExternalInput":
            if name != partition_name:
                in_names.append(name)
        elif alloc.kind == "ExternalOutput":
            shape = tuple(alloc.tensor_shape)
            dtype = mybir.dt.np(alloc.dtype)
            out_names.append(name)
            out_avals.append(jax.core.ShapedArray(shape, dtype))
            zero_outs.append(np.zeros(shape, dtype))
    n_params = len(in_names)
    all_in_names = list(in_names) + list(out_names)
    if partition_name is not None:
        all_in_names.append(partition_name)

    def _body(*args):
        operands = list(args)
        if partition_name is not None:
            operands.append(partition_id_tensor())
        outs = _bass_exec_p.bind(
            *operands,
            out_avals=tuple(out_avals),
            in_names=tuple(all_in_names),
            out_names=tuple(out_names),
            lowering_input_output_aliases=(),
            sim_require_finite=True,
            sim_require_nnan=True,
            nc=nc,
        )
        return tuple(outs)

    devices = jax.devices()[:n_cores]
    mesh = Mesh(np.asarray(devices), ("core",))
    in_specs = (PartitionSpec("core"),) * (n_params + len(out_names))
    out_specs = (PartitionSpec("core"),) * len(out_names)
    fn = jax.jit(
        shard_map(
            _body, mesh=mesh, in_specs=in_specs, out_specs=out_specs,
            check_rep=False,
        ),
        keep_unused=True,
    )
    sharding = NamedSharding(mesh, PartitionSpec("core"))
    zeros_dev = [
        jax.device_put(
            np.zeros((n_cores * z.shape[0], *z.shape[1:]), z.dtype), sharding
        )
        for z in zero_outs
    ]

    def run(in_maps):
        per_core = [[np.asarray(m[n]) for n in in_names] for m in in_maps]
        concat_in = [
            np.concatenate([per_core[c][i] for c in range(n_cores)], axis=0)
            for i in range(n_params)
        ]
        args = [jax.device_put(a, sharding) for a in concat_in] + zeros_dev
        out = fn(*args)
        jax.block_until_ready(out)
        return [
            {
                n: np.asarray(out[i]).reshape(n_cores, *out_avals[i].shape)[c]
                for i, n in enumerate(out_names)
            }
            for c in range(n_cores)
        ]

    return run, fn, in_names, out_names, out_avals, sharding


_RUNNER = None


def _in_maps_from_inputs(inputs):
    import ml_dtypes

    ident = np.eye(P, dtype=ml_dtypes.bfloat16)
    maps = []
    for b in range(N_CORES):
        m = {
            "xq": np.ascontiguousarray(np.asarray(inputs["x_q"][b], np.float32)),
            "xk": np.ascontiguousarray(np.asarray(inputs["x_k"][b], np.float32)),
            "xv": np.ascontiguousarray(np.asarray(inputs["x_v"][b], np.float32)),
            "wq": np.asarray(inputs["Wq"], np.float32),
            "wk": np.asarray(inputs["Wk"], np.float32),
            "wv": np.asarray(inputs["Wv"], np.float32),
            "wo": np.asarray(inputs["Wo"], np.float32),
            "bk": np.asarray(inputs["bk"], np.float32),
            "bv": np.asarray(inputs["bv"], np.float32),
            "bo": np.asarray(inputs["bo"], np.float32),
            "ident": ident,
        }
        maps.append(m)
    return maps


def kernel(**inputs) -> np.ndarray:
    global _RUNNER
    if _RUNNER is None:
        nc = build_nc()
        _RUNNER = make_runner(nc)[0]
    in_maps = _in_maps_from_inputs(inputs)
    results = _RUNNER(in_maps)
    out = np.stack([results[b]["y"] for b in range(N_CORES)], axis=0)
    return out.astype(np.float32)


# revision 12
# speedup vs baseline: 2.2719x; 1.1095x over previous
"""MultiHeadAttention (CLUSTERING softmax over query axis) on 8 Trainium2 cores.

Sharding: batch B=8, one batch element per NeuronCore (pure data parallel,
no collectives).

Per-core computation (L=1024, D=1024, H=16, HD=64):
  QT = (x_q @ Wq)^T            [d, l]   (bq dropped: cancels in softmax over q)
  KT = (x_k @ Wk + bk)^T       [d, l]
  V  = x_v @ Wv + bv           [l, d]
  per head h: ST_h[k, q] = QT_h . KT_h  (contraction over hd=64)
  E = exp(ST / 32)  with fused row-sums over q (free axis)
  r = 1/sums; V'_h[k, :] = V_h[k, :] * r_h[k]   (normalizer folded into V)
  OT_h[d, q] = sum_k V'_h[k, d] * E_h[k, q]
  y = OT^T @ Wo + bo           [l, d]

All matmuls bf16 with fp32 PSUM accumulation. x^T via PE transpose-mode
(bf16, batched per-c-tile into one PSUM tile). exp on ScalarE from PSUM.
Head pairs are packed onto the 128-wide PE array (row-tiling for scores,
col-tiling for A.V) since HD=64.
"""

import math
from contextlib import ExitStack, nullcontext

import numpy as np

import concourse.bass as bass
import concourse.tile as tile
from concourse import mybir
from concourse.bass import ts

F32 = mybir.dt.float32
BF16 = mybir.dt.bfloat16
EXP = mybir.ActivationFunctionType.Exp
COPY = mybir.ActivationFunctionType.Copy

L = 1024
D = 1024
P = 128
NT = 8  # 1024 / 128
N_CORES = 8
SCALE = 1.0 / math.sqrt(D)


# ---------------------------------------------------------------------------
# Workaround: this walrus build supports very few sync-wait commands per
# instruction. Tile's kernel-tail drain / barriers can carry more. Move
# excess waits onto same-engine NOPs inserted immediately before (engines
# execute their stream in order, so this preserves semantics).
def split_excess_waits(nc):
    f = nc.m.functions[0]
    ctr = 0
    for b in f.blocks:
        insts = b.instructions
        i = 0
        while i < len(insts):
            inst = insts[i]
            si = inst.sync_info
            limit = 0 if "Drain" in type(inst).__name__ else 1
            if si is not None and si.on_wait and len(si.on_wait) > limit:
                waits = list(si.on_wait)
                keep = waits[-limit:] if limit else []
                extra = waits[: len(waits) - limit]
                pos = i
                for j in range(0, len(extra), 1):
                    nop = mybir.InstNoOp(name=f"waitsplit-{ctr}", ins=[], outs=[])
                    ctr += 1
                    nop.engine = inst.engine
                    nop.bass_nofuse = True
                    nop.sync_info = mybir.SyncInfo(
                        on_wait=[extra[j]], on_update=[]
                    )
                    insts.insert(pos, nop)
                    pos += 1
                    i += 1
                inst.sync_info = mybir.SyncInfo(
                    on_wait=keep, on_update=list(si.on_update)
                )
            i += 1


# ---------------------------------------------------------------------------
def _emit_body(nc, tc, ctx, t):
    persist = ctx.enter_context(tc.tile_pool(name="persist", bufs=1))
    projp = ctx.enter_context(tc.tile_pool(name="projp", bufs=2, space="PSUM"))

    # ---- constants -------------------------------------------------------
    ones_t = persist.tile([1, P], BF16, name="ones")
    nc.vector.memset(ones_t[:], 1.0)
    bk_sb = persist.tile([P, NT], F32, name="bk")
    nc.sync.dma_start(bk_sb[:], t["bk"].rearrange("(a p) -> p a", p=P))
    bo_bf = persist.tile([1, D], BF16, name="bo")

    v_sb = [persist.tile([P, D], BF16, name=f"v{i}") for i in range(NT)]
    ot_sb = [persist.tile([P, D], BF16, name=f"ot{i}") for i in range(NT)]
    wo_bf = [persist.tile([P, D], BF16, name=f"wo{i}") for i in range(NT)]

    with ExitStack() as xw_ctx:
        xw = xw_ctx.enter_context(tc.tile_pool(name="xw", bufs=1))
        wq_bf = [xw.tile([P, D], BF16, name=f"wq{i}") for i in range(NT)]
        wk_bf = [xw.tile([P, D], BF16, name=f"wk{i}") for i in range(NT)]
        xqT = [xw.tile([P, D], BF16, name=f"xqT{i}") for i in range(NT)]
        xkT = [xw.tile([P, D], BF16, name=f"xkT{i}") for i in range(NT)]

        with ExitStack() as vw_ctx:
            vw = vw_ctx.enter_context(tc.tile_pool(name="vw", bufs=1))
            wv_bf = [vw.tile([P, D], BF16, name=f"wv{i}") for i in range(NT)]
            xvT = [vw.tile([P, D], BF16, name=f"xvT{i}") for i in range(NT)]
            ident = vw.tile([P, P], BF16, name="ident")
            nc.sync.dma_start(ident[:], t["ident"][:, :])
            bv_bf = vw.tile([1, D], BF16, name="bv")

            with ExitStack() as ph_a:
                stgp = ph_a.enter_context(tc.tile_pool(name="stgp", bufs=5))
                xbfp = ph_a.enter_context(tc.tile_pool(name="xbfp", bufs=1))
                tpp = ph_a.enter_context(
                    tc.tile_pool(name="tpp", bufs=2, space="PSUM")
                )

                bstg = stgp.tile([P, D], F32, name="stg")
                nc.sync.dma_start(bstg[0:1, :], t["bv"][None, :])
                nc.vector.tensor_copy(bv_bf[:], bstg[0:1, :])
                bstg2 = stgp.tile([P, D], F32, name="stg")
                nc.sync.dma_start(bstg2[0:1, :], t["bo"][None, :])
                nc.vector.tensor_copy(bo_bf[:], bstg2[0:1, :])

                def wpath(w_tiles, wdram):
                    for i in range(NT):
                        stg = stgp.tile([P, D], F32, name="stg")
                        nc.sync.dma_start(stg[:], wdram[ts(i, P), :])
                        nc.vector.tensor_copy(w_tiles[i][:], stg[:])

                def xpath(xT_tiles, xdram, act_copies):
                    for half in range(2):
                        xbfs = []
                        for lt in range(4 * half, 4 * half + 4):
                            stg = stgp.tile([P, D], F32, name="stg")
                            nc.sync.dma_start(stg[:], xdram[ts(lt, P), :])
                            xbf = xbfp.tile([P, D], BF16, name=f"xbf{lt}")
                            nc.vector.tensor_copy(xbf[:], stg[:])
                            xbfs.append(xbf)
                        for ct in range(NT):
                            tp = tpp.tile([P, 512], BF16, name="tp")
                            for i, lt in enumerate(range(4 * half, 4 * half + 4)):
                                nc.tensor.transpose(
                                    tp[:, ts(i, P)],
                                    xbfs[i][:, ts(ct, P)],
                                    ident[:],
                                )
                            dst = xT_tiles[ct][:, 512 * half : 512 * half + 512]
                            if act_copies:
                                nc.scalar.activation(dst, tp[:], COPY)
                            else:
                                nc.vector.tensor_copy(dst, tp[:])

                # V path first: its projection fills the PE while q/k load.
                xpath(xvT, t["xv"], act_copies=True)
                wpath(wv_bf, t["wv"])

                # ---- V projection: V[l, d] = x_v @ Wv + bv --------------
                for lt in range(NT):
                    ps = [projp.tile([P, 512], F32, name="pp") for _ in range(2)]
                    for ct in range(NT):
                        for dc in range(2):
                            nc.tensor.matmul(
                                ps[dc][:],
                                xvT[ct][:, ts(lt, P)],
                                wv_bf[ct][:, ts(dc, 512)],
                                start=(ct == 0),
                                stop=False,
                            )
                    for dc in range(2):
                        nc.tensor.matmul(
                            ps[dc][:],
                            ones_t[0:1, 0:P],
                            bv_bf[0:1, ts(dc, 512)],
                            start=False,
                            stop=True,
                        )
                        nc.vector.tensor_copy(v_sb[lt][:, ts(dc, 512)], ps[dc][:])

                xpath(xqT, t["xq"], act_copies=True)
                wpath(wq_bf, t["wq"])
                xpath(xkT, t["xk"], act_copies=True)
                wpath(wk_bf, t["wk"])
                wpath(wo_bf, t["wo"])

        # ---- attention pipeline over head pairs -------------------------
        qtkt = xw_ctx.enter_context(tc.tile_pool(name="qtkt", bufs=2))
        epool = xw_ctx.enter_context(tc.tile_pool(name="epool", bufs=3))
        sums = xw_ctx.enter_context(tc.tile_pool(name="sums", bufs=4))
        vppool = xw_ctx.enter_context(tc.tile_pool(name="vppool", bufs=2))
        ypool = xw_ctx.enter_context(tc.tile_pool(name="ypool", bufs=1))
        stp0 = xw_ctx.enter_context(tc.tile_pool(name="stp0", bufs=1, space="PSUM"))
        stp1 = xw_ctx.enter_context(tc.tile_pool(name="stp1", bufs=1, space="PSUM"))
        avp = xw_ctx.enter_context(tc.tile_pool(name="avp", bufs=1, space="PSUM"))

        def emit_scores(hp, qt, kt_t):
            e0 = epool.tile([P, NT, L], BF16, name="e")
            e1 = epool.tile([P, NT, L], BF16, name="e")
            s0 = sums.tile([P, NT], F32, name="esum")
            s1 = sums.tile([P, NT], F32, name="esum")
            for kt in range(NT):
                st0 = stp0.tile([P, L], F32, name="st0")
                st1 = stp1.tile([P, L], F32, name="st1")
                for qc in range(2):
                    nc.tensor.matmul(
                        st0[:, ts(qc, 512)],
                        kt_t[0:64, ts(kt, P)],
                        qt[0:64, ts(qc, 512)],
                        start=True,
                        stop=True,
                    )
                for qc in range(2):
                    nc.tensor.matmul(
                        st1[:, ts(qc, 512)],
                        kt_t[64:128, ts(kt, P)],
                        qt[64:128, ts(qc, 512)],
                        start=True,
                        stop=True,
                    )
                nc.scalar.activation(
                    e0[:, kt, :], st0[:], EXP, scale=SCALE,
                    accum_out=s0[:, kt : kt + 1],
                )
                nc.scalar.activation(
                    e1[:, kt, :], st1[:], EXP, scale=SCALE,
                    accum_out=s1[:, kt : kt + 1],
                )
            r0 = sums.tile([P, NT], F32, name="r")
            r1 = sums.tile([P, NT], F32, name="r")
            nc.vector.reciprocal(r0[:], s0[:])
            nc.vector.reciprocal(r1[:], s1[:])
            vp = vppool.tile([P, NT, P], BF16, name="vp")
            for kt in range(NT):
                nc.vector.tensor_scalar_mul(
                    vp[:, kt, 0:64],
                    v_sb[kt][:, hp * P : hp * P + 64],
                    r0[:, kt : kt + 1],
                )
                nc.vector.tensor_scalar_mul(
                    vp[:, kt, 64:128],
                    v_sb[kt][:, hp * P + 64 : hp * P + 128],
                    r1[:, kt : kt + 1],
                )
            return (hp, e0, e1, vp)

        def emit_av(prev):
            # One PSUM bank per accumulation group: pending groups must not
            # share a bank (start_tensor_calc zeroes the whole zero-region).
            hp, e0, e1, vp = prev
            for qc in range(2):
                avA = avp.tile([P, 512], F32, name="avA")
                avB = avp.tile([P, 512], F32, name="avB")
                for kt in range(NT):
                    nc.tensor.matmul(
                        avA[0:64, :],
                        vp[:, kt, 0:64],
                        e0[:, kt, ts(qc, 512)],
                        start=(kt == 0),
                        stop=(kt == NT - 1),
                    )
                    nc.tensor.matmul(
                        avB[64:128, :],
                        vp[:, kt, 64:128],
                        e1[:, kt, ts(qc, 512)],
                        start=(kt == 0),
                        stop=(kt == NT - 1),
                    )
                nc.vector.tensor_copy(
                    ot_sb[hp][0:64, ts(qc, 512)], avA[0:64, :]
                )
                nc.vector.tensor_copy(
                    ot_sb[hp][64:128, ts(qc, 512)], avB[64:128, :]
                )

        def emit_proj(hp, w_bf, xT, out_tag):
            out_t = qtkt.tile([P, L], BF16, name=out_tag)
            ps = [projp.tile([P, 512], F32, name="pp") for _ in range(2)]
            for ct in range(NT):
                for lc in range(2):
                    nc.tensor.matmul(
                        ps[lc][:],
                        w_bf[ct][:, ts(hp, P)],
                        xT[ct][:, ts(lc, 512)],
                        start=(ct == 0),
                        stop=(ct == NT - 1),
                    )
            for lc in range(2):
                if out_tag == "kt":
                    nc.vector.tensor_scalar_add(
                        out_t[:, ts(lc, 512)], ps[lc][:], bk_sb[:, hp : hp + 1]
                    )
                else:
                    nc.vector.tensor_copy(out_t[:, ts(lc, 512)], ps[lc][:])
            return out_t

        ypart = [ypool.tile([P, D], F32, name=f"yp{i}") for i in range(NT)]

        def outproj_batch1():
            # contract pairs 0..6 into y partials while pair 7 is in flight
            for lt in range(NT):
                ps = [projp.tile([P, 512], F32, name="pp") for _ in range(2)]
                for dt in range(NT - 1):
                    for nc2 in range(2):
                        nc.tensor.matmul(
                            ps[nc2][:],
                            ot_sb[dt][:, ts(lt, P)],
                            wo_bf[dt][:, ts(nc2, 512)],
                            start=(dt == 0),
                            stop=(dt == NT - 2),
                        )
                for nc2 in range(2):
                    nc.vector.tensor_copy(
                        ypart[lt][:, ts(nc2, 512)], ps[nc2][:]
                    )

        def outproj_batch2():
            for lt in range(NT):
                ps = [projp.tile([P, 512], F32, name="pp") for _ in range(2)]
                for nc2 in range(2):
                    nc.tensor.matmul(
                        ps[nc2][:],
                        ot_sb[NT - 1][:, ts(lt, P)],
                        wo_bf[NT - 1][:, ts(nc2, 512)],
                        start=True,
                        stop=False,
                    )
                    nc.tensor.matmul(
                        ps[nc2][:],
                        ones_t[0:1, 0:P],
                        bo_bf[0:1, ts(nc2, 512)],
                        start=False,
                        stop=True,
                    )
                for nc2 in range(2):
                    nc.vector.tensor_tensor(
                        ypart[lt][:, ts(nc2, 512)],
                        ps[nc2][:],
                        ypart[lt][:, ts(nc2, 512)],
                        mybir.AluOpType.add,
                    )
                nc.sync.dma_start(t["y"][ts(lt, P), :], ypart[lt][:])

        prev = None
        for hp in range(NT):
            qt = emit_proj(hp, wq_bf, xqT, "qt")
            kt_t = emit_proj(hp, wk_bf, xkT, "kt")
            if prev is not None:
                emit_av(prev)
            if hp == NT - 1:
                outproj_batch1()
            prev = emit_scores(hp, qt, kt_t)
        emit_av(prev)
        outproj_batch2()


def build_nc(looped=False, reps=None, do_split=True):
    nc = bass.Bass("TRN2", debug=False, num_devices=N_CORES, num_swdge_queues=4)
    t = {}
    for name in ("xq", "xk", "xv"):
        t[name] = nc.dram_tensor(name, [L, D], F32, kind="ExternalInput")
    for name in ("wq", "wk", "wv", "wo"):
        t[name] = nc.dram_tensor(name, [D, D], F32, kind="ExternalInput")
    for name in ("bk", "bv", "bo"):
        t[name] = nc.dram_tensor(name, [D], F32, kind="ExternalInput")
    t["ident"] = nc.dram_tensor("ident", [P, P], BF16, kind="ExternalInput")
    t["y"] = nc.dram_tensor("y", [L, D], F32, kind="ExternalOutput")

    with tile.TileContext(nc) as tc:
        if reps is not None:
            loop_cm = tc.For_i(0, reps, 1)
        else:
            loop_cm = nullcontext()
        with loop_cm:
            with ExitStack() as ctx:
                _emit_body(nc, tc, ctx, t)

    if do_split:
        split_excess_waits(nc)
    return nc


# ---------------------------------------------------------------------------
# Runner: mirrors bass2jax.run_bass_via_pjrt's multi-core path, but keeps a
# reusable jitted callable (no donation) so repeated kernel() calls don't
# recompile.
def make_runner(nc, n_cores=N_CORES):
    import jax
    from jax.sharding import Mesh, NamedSharding, PartitionSpec
    from jax.experimental.shard_map import shard_map
    from concourse import bass2jax
    from concourse.bass2jax import _bass_exec_p, partition_id_tensor

    bass2jax.install_neuronx_cc_hook()

    partition_name = (
        nc.partition_id_tensor.name if nc.partition_id_tensor else None
    )
    in_names, out_names, out_avals, zero_outs = [], [], [], []
    for alloc in nc.m.functions[0].allocations:
        if not isinstance(alloc, mybir.MemoryLocationSet):
            continue
        name = alloc.memorylocations[0].name
        if alloc.kind == "ExternalInput":
            if name != partition_name:
                in_names.append(name)
        elif alloc.kind == "ExternalOutput":
            shape = tuple(alloc.tensor_shape)
            dtype = mybir.dt.np(alloc.dtype)
            out_names.append(name)
            out_avals.append(jax.core.ShapedArray(shape, dtype))
            zero_outs.append(np.zeros(shape, dtype))
    n_params = len(in_names)
    all_in_names = list(in_names) + list(out_names)
    if partition_name is not None:
        all_in_names.append(partition_name)

    def _body(*args):
        operands = list(args)
        if partition_name is not None:
            operands.append(partition_id_tensor())
        outs = _bass_exec_p.bind(
            *operands,
            out_avals=tuple(out_avals),
            in_names=tuple(all_in_names),
            out_names=tuple(out_names),
            lowering_input_output_aliases=(),
            sim_require_finite=True,
            sim_require_nnan=True,
            nc=nc,
        )
        return tuple(outs)

    devices = jax.devices()[:n_cores]
    mesh = Mesh(np.asarray(devices), ("core",))
    in_specs = (PartitionSpec("core"),) * (n_params + len(out_names))
    out_specs = (PartitionSpec("core"),) * len(out_names)
    fn = jax.jit(
        shard_map(
            _body, mesh=mesh, in_specs=in_specs, out_specs=out_specs,
            check_rep=False,
        ),
        keep_unused=True,
    )
    sharding = NamedSharding(mesh, PartitionSpec("core"))
    zeros_dev = [
        jax.device_put(
            np.zeros((n_cores * z.shape[0], *z.shape[1:]), z.dtype), sharding
        )
        for z in zero_outs
    ]

    def run(in_maps):
        per_core = [[np.asarray(m[n]) for n in in_names] for m in in_maps]
        concat_in = [
            np.concatenate([per_core[c][i] for c in range(n_cores)], axis=0)
            for i in range(n_params)
        ]
        args = [jax.device_put(a, sharding) for a in concat_in] + zeros_dev
        out = fn(*args)
        jax.block_until_ready(out)
        return [
            {
                n: np.asarray(out[i]).reshape(n_cores, *out_avals[i].shape)[c]
                for i, n in enumerate(out_names)
            }
            for c in range(n_cores)
        ]

    return run, fn, in_names, out_names, out_avals, sharding


_RUNNER = None


def _in_maps_from_inputs(inputs):
    import ml_dtypes

    ident = np.eye(P, dtype=ml_dtypes.bfloat16)
    maps = []
    for b in range(N_CORES):
        m = {
            "xq": np.ascontiguousarray(np.asarray(inputs["x_q"][b], np.float32)),
            "xk": np.ascontiguousarray(np.asarray(inputs["x_k"][b], np.float32)),
            "xv": np.ascontiguousarray(np.asarray(inputs["x_v"][b], np.float32)),
            "wq": np.asarray(inputs["Wq"], np.float32),
            "wk": np.asarray(inputs["Wk"], np.float32),
            "wv": np.asarray(inputs["Wv"], np.float32),
            "wo": np.asarray(inputs["Wo"], np.float32),
            "bk": np.asarray(inputs["bk"], np.float32),
            "bv": np.asarray(inputs["bv"], np.float32),
            "bo": np.asarray(inputs["bo"], np.float32),
            "ident": ident,
        }
        maps.append(m)
    return maps


def kernel(**inputs) -> np.ndarray:
    global _RUNNER
    if _RUNNER is None:
        nc = build_nc()
        _RUNNER = make_runner(nc)[0]
    in_maps = _in_maps_from_inputs(inputs)
    results = _RUNNER(in_maps)
    out = np.stack([results[b]["y"] for b in range(N_CORES)], axis=0)
    return out.astype(np.float32)
